# revision 11
# baseline (speedup 1.0000x reference)
# Relational GCN message-passing layer (MolGAN-style) on 8 Trainium2 NeuronCores.
#
#   x_new[s,i,b] = tanh( sum_c norm[s,i,c] * sum_{j,a} A[s,i,j,c] x[s,j,a] W[a,b,c]
#                        + (x @ theta_root)[s,i,b] )
#   norm[s,i,c] = 1 / (sum_j A[s,i,j,c] + eps)        (c < 4; channel 4 dropped)
#
# Sharding: data-parallel over the batch dim s — 16 batches / 8 cores = 2 per core.
# Each core streams its 42 MB A-slice once (memory-bound target ~117 us/core at
# ~358 GB/s HBM-per-NC).
#
# Per-core dataflow, per (s, i_block) slab A[s, i_block, :, :] = [128, 1024, 5]:
#   1. SWDGE DMA loads the slab contiguously, casting fp32 -> fp16 in flight.
#   2. PE transposes 128x128 tiles (j on partitions) into fp16 PSUM banks,
#      packed 8 tiles/bank; DVE/ACT copy banks to SBUF.
#   3. Stage-1 GEMM per relation c: m~[i, 0:129] = sum_jb AT[c,jb].T @ x~[jb]
#      where x~ has a ones column appended -> column 128 is the degree row-sum
#      (the normalizer) for free.
#   4. norm = 1/rowsum (DVE reciprocal), applied as the per-partition scale of
#      the ACT PSUM->SBUF copy (out = psum * norm, cast to fp16).
#   5. m tiles transposed back (PE) so stage-2 contracts over (c,a):
#      out[i,b] = sum_c mT_c.T @ W_c + xT.T @ theta  (5 accumulating matmuls).
#   6. tanh on ACT (PSUM -> SBUF fp32), HWDGE DMA out.

import os
from contextlib import ExitStack

import numpy as np

import concourse.tile as tile
from concourse import bacc, mybir
from concourse.bass_utils import run_bass_kernel_spmd
from concourse.masks import make_identity

S, N, C5, R, CIN, COUT = 16, 1024, 5, 4, 128, 128
NCORES = 8
SPC = S // NCORES  # batches per core
NB = N // 128      # 128-row node blocks
XW = CIN + 2       # x~ row stride: 128 data + 1 ones + 1 pad (4B alignment)

F16 = mybir.dt.float16
F32 = mybir.dt.float32


def _kernel_body(tc, bench_iters=1):
    nc = tc.nc
    A = nc.dram_tensor("A", (SPC, N, N, C5), F32, kind="ExternalInput").ap()
    x = nc.dram_tensor("x", (SPC, N, CIN), F32, kind="ExternalInput").ap()
    w = nc.dram_tensor("weight", (CIN, COUT, R), F32, kind="ExternalInput").ap()
    th = nc.dram_tensor("theta_root", (CIN, COUT), F32, kind="ExternalInput").ap()
    y = nc.dram_tensor("y", (SPC, N, COUT), F32, kind="ExternalOutput").ap()

    with ExitStack() as ctx:
        # bufs tuned on HW: slabs=3/atp=2 measured ~10 us/iter faster than
        # slabs=4/atp=3 (3 outstanding 2.56 MB A-streams degrade HBM locality).
        consts = ctx.enter_context(tc.tile_pool(name="consts", bufs=1))
        slabs = ctx.enter_context(tc.tile_pool(name="slabs", bufs=2))
        atp = ctx.enter_context(tc.tile_pool(name="atp", bufs=2))
        small = ctx.enter_context(tc.tile_pool(name="small", bufs=3))
        outp = ctx.enter_context(tc.tile_pool(name="outp", bufs=2))
        ptp = ctx.enter_context(tc.tile_pool(name="ptp", bufs=2, space="PSUM"))
        pm = ctx.enter_context(tc.tile_pool(name="pm", bufs=2, space="PSUM"))
        pmt = ctx.enter_context(tc.tile_pool(name="pmt", bufs=2, space="PSUM"))
        po = ctx.enter_context(tc.tile_pool(name="po", bufs=2, space="PSUM"))

        ident = consts.tile([128, 128], F16)
        make_identity(nc, ident)

        # weight [a,b,c] -> w2 [a,c,b] fp16 so stage-2 rhs streams contiguously
        wtmp = consts.tile([128, COUT * R], F16)
        nc.gpsimd.dma_start(out=wtmp, in_=w.rearrange("a b c -> a (b c)"))
        w2 = consts.tile([128, R, COUT], F16)
        wv = wtmp.rearrange("a (b c) -> a b c", c=R)
        for c in range(R):
            nc.vector.tensor_copy(out=w2[:, c, :], in_=wv[:, :, c])
        th16 = consts.tile([128, COUT], F16)
        nc.gpsimd.dma_start(out=th16, in_=th)

        # x~ tiles: [j, 0:128]=x (fp16), col 128 = 1.0 (rowsum probe)
        xe = consts.tile([128, SPC * NB, XW], F16)
        nc.vector.memset(xe[:, :, CIN], 1.0)
        for s in range(SPC):
            for jb in range(NB):
                nc.gpsimd.dma_start(
                    out=xe[:, s * NB + jb, :CIN],
                    in_=x[s, jb * 128 : (jb + 1) * 128, :],
                )
        # xT tiles [a, i] for the theta_root term
        xT = consts.tile([128, SPC * NB, CIN], F16)
        for k in range(SPC * NB):
            pt = pmt.tile([128, 128], F16, tag="mt")
            nc.tensor.transpose(pt, xe[:, k, :CIN], ident)
            nc.vector.tensor_copy(out=xT[:, k, :], in_=pt)

        def transpose_group(slab_t, at_t, p):
            # Transpose 8 [128,128] tiles (jb in {2p, 2p+1} x c in 0..3) into one
            # fp16 PSUM bank, then one wide copy to SBUF.
            ps = ptp.tile([128, 1024], F16, tag="tp")
            for q in range(2):
                jb = 2 * p + q
                for c in range(R):
                    col = q * 512 + c * 128
                    nc.tensor.transpose(
                        ps[:, col : col + 128],
                        slab_t[:, jb * 128 : (jb + 1) * 128, c],
                        ident,
                    )
            dst = at_t[:, p * 1024 : (p + 1) * 1024]
            if p % 2 == 0:
                nc.vector.tensor_copy(out=dst, in_=ps)
            else:
                nc.scalar.copy(out=dst, in_=ps)

        def stage1(si, at_t, c):
            # m~[i, 0:129] = sum_jb AT[c,jb].T @ x~[jb];  col 128 = degree rowsum
            m = pm.tile([128, CIN + 1], F32, tag="m")
            for jb in range(NB):
                nc.tensor.matmul(
                    m,
                    lhsT=at_t[:, jb * 512 + c * 128 : jb * 512 + (c + 1) * 128],
                    rhs=xe[:, si * NB + jb, : CIN + 1],
                    start=(jb == 0),
                    stop=(jb == NB - 1),
                )
            nrm = small.tile([128, 1], F32, tag="norm")
            nc.vector.reciprocal(nrm, m[:, CIN : CIN + 1])
            mn = small.tile([128, CIN], F16, tag="mn")
            nc.scalar.mul(mn, m[:, :CIN], nrm)  # psum * norm -> fp16 SBUF
            pt = pmt.tile([128, 128], F16, tag="mt")
            nc.tensor.transpose(pt, mn, ident)
            mt = small.tile([128, CIN], F16, tag="mts")
            nc.vector.tensor_copy(out=mt, in_=pt)
            return mt

        def stage2(si, ib, mts):
            out_ps = po.tile([128, COUT], F32, tag="o")
            for c in range(R):
                nc.tensor.matmul(
                    out_ps, lhsT=mts[c], rhs=w2[:, c, :], start=(c == 0), stop=False
                )
            nc.tensor.matmul(
                out_ps, lhsT=xT[:, si * NB + ib, :], rhs=th16, start=False, stop=True
            )
            ot = outp.tile([128, COUT], F32, tag="out")
            nc.scalar.activation(ot, out_ps, mybir.ActivationFunctionType.Tanh)
            nc.sync.dma_start(out=y[si, ib * 128 : (ib + 1) * 128, :], in_=ot)

        # Main loop, software-pipelined: transposes of slab t interleave with
        # stage-1/2 matmuls of slab t-1 so the PE sees a steady matmul mix.
        def main_pipeline():
            prev = None
            si = ib = 0
            for t in range(SPC * NB + 1):
                if t < SPC * NB:
                    si, ib = divmod(t, NB)
                    slab_t = slabs.tile([128, N, C5], F16, tag="slab")
                    # Chunked load: transpose group p only needs j-columns
                    # [256p, 256p+256), so 4 sub-DMAs (640 KB each, 5.1 KB
                    # contiguous per partition-row) let the PE start on the
                    # first quarter while the rest streams in.
                    for p4 in range(4):
                        nc.gpsimd.dma_start(
                            out=slab_t[:, p4 * 256 : (p4 + 1) * 256, :],
                            in_=A[
                                si,
                                ib * 128 : (ib + 1) * 128,
                                p4 * 256 : (p4 + 1) * 256,
                                :,
                            ],
                        )
                    at_t = atp.tile([128, NB * R * 128], F16, tag="at")
                mts = []
                for p in range(4):
                    if t < SPC * NB:
                        transpose_group(slab_t, at_t, p)
                    if prev is not None:
                        mts.append(stage1(prev[0], prev[2], p))
                if prev is not None:
                    stage2(prev[0], prev[1], mts)
                prev = (si, ib, at_t) if t < SPC * NB else None

        if bench_iters > 1:
            # Bench mode: repeat the whole pipeline on-device so steady-state
            # HW time can be resolved through the ~88 ms axon dispatch noise.
            with tc.For_i(
                0,
                bench_iters,
                1,
                hint_engines=(
                    mybir.EngineType.PE,
                    mybir.EngineType.DVE,
                    mybir.EngineType.Activation,
                    mybir.EngineType.Pool,
                ),
            ):
                main_pipeline()
        else:
            main_pipeline()


_CACHE = {}


def build_nc(bench_iters=1):
    nc = bacc.Bacc(
        "TRN2", target_bir_lowering=False, debug=False, num_devices=NCORES
    )
    with tile.TileContext(nc) as tc:
        _kernel_body(tc, bench_iters)
    nc.compile()  # Bacc register-allocation / DCE pass
    return nc


def _get_nc():
    if "nc" not in _CACHE:
        _CACHE["nc"] = build_nc(1)
    return _CACHE["nc"]


LAST = None  # BassKernelResults of the most recent run (for profiling)


def kernel(A, x, weight, theta_root):
    global LAST
    A = np.ascontiguousarray(np.asarray(A), dtype=np.float32)
    x = np.ascontiguousarray(np.asarray(x), dtype=np.float32)
    weight = np.ascontiguousarray(np.asarray(weight), dtype=np.float32)
    theta_root = np.ascontiguousarray(np.asarray(theta_root), dtype=np.float32)

    # The axon NTFF trace hook isn't shipped in this container; make sure a
    # stray BASS_TRACE=1 in the environment can't divert run_bass_kernel_spmd
    # into the (crashing) trace path.
    os.environ["BASS_NEVER_TRACE"] = "1"

    nc = _get_nc()
    in_maps = []
    for k in range(NCORES):
        sl = slice(k * SPC, (k + 1) * SPC)
        in_maps.append(
            {
                "A": np.ascontiguousarray(A[sl]),
                "x": np.ascontiguousarray(x[sl]),
                "weight": weight,
                "theta_root": theta_root,
            }
        )
    res = run_bass_kernel_spmd(nc, in_maps, core_ids=list(range(NCORES)))
    LAST = res
    return np.concatenate([r["y"] for r in res.results], axis=0)
